# revision 3
# baseline (speedup 1.0000x reference)
"""Causal self-attention (B=4, T=2048, C=1024, H=16) on 8 TRN2 NeuronCores.

Sharding: core = 2*b + hg  (b = batch 0..3, hg = head-group 0..1, 8 heads each).
Each core computes, for its (b, hg):
  - qkv^T for its 8 heads via fp32r matmuls (x^T resident, W chunks stationary)
  - causal attention per head in scores-transposed orientation [keys, queries]
    with exp on ACT (no max subtraction; scores are O(1)), denominators via an
    appended ones-column in the AV matmul, and deferred normalization
  - partial output projection out^T = Wproj_loc^T @ y^T (+ bias on hg==0)
  - pairwise ReduceScatter (cores 2b, 2b+1) so each core returns half the
    out^T rows, already summed.
Host side transposes x per batch on the way in and reassembles/transposes the
output on the way out. All matmuls run in float32r (~1e-4 rel err, full PE rate).
"""
import numpy as np

import concourse.bass as bass
from concourse import bacc, mybir
from concourse.tile import TileContext
from concourse.bass_utils import run_bass_kernel_spmd

dt = mybir.dt
AF = mybir.ActivationFunctionType

B, T, C, H = 4, 2048, 1024, 16
D = 64              # head dim
HL = 8              # heads per core
CL = HL * D         # 512 local channels
NQ = T // 512       # 4 query chunks of 512
NT = T // 128       # 16 key/time chunks of 128
SCALE = 1.0 / np.sqrt(D)

_CACHE = {}


def _build_nc():
    nc = bacc.Bacc("TRN2", target_bir_lowering=False, debug=False)

    xT_e = nc.declare_dram_parameter("xT", [C, T], dt.float32r, isOutput=False)
    wqk_e = nc.declare_dram_parameter("wqk", [C, 2 * CL], dt.float32r, isOutput=False)
    wv_e = nc.declare_dram_parameter("wv", [C, CL], dt.float32r, isOutput=False)
    bqk_e = nc.declare_dram_parameter("bqk", [128, 8], dt.float32, isOutput=False)
    bvr_e = nc.declare_dram_parameter("bvr", [1, CL], dt.float32r, isOutput=False)
    wp_e = nc.declare_dram_parameter("wproj", [CL, C], dt.float32r, isOutput=False)
    bp_e = nc.declare_dram_parameter("bproj", [128, 8], dt.float32, isOutput=False)
    out_e = nc.declare_dram_parameter("out", [CL, T], dt.float32, isOutput=True)

    RG = [[0, 1], [2, 3], [4, 5], [6, 7]]

    with TileContext(nc) as tc, nc.allow_low_precision("fp32r intermediates by design"):
        from contextlib import ExitStack
        with ExitStack() as top:
            p_cst = top.enter_context(tc.tile_pool(name="cst", bufs=1))
            p_qk = top.enter_context(tc.tile_pool(name="qk", bufs=8))
            p_v = top.enter_context(tc.tile_pool(name="v", bufs=16))

            ones_f = p_cst.tile([128, 128], dt.float32)
            nc.gpsimd.memset(ones_f[:], 1.0)
            ones_row = p_cst.tile([1, 128], dt.float32r)
            nc.vector.tensor_copy(ones_row[:], ones_f[0:1, :])
            ones_rr = p_cst.tile([128, 64], dt.float32r)
            nc.vector.tensor_copy(ones_rr[:], ones_f[:, 0:64])
            bqk_sb = p_cst.tile([128, 8], dt.float32)
            nc.sync.dma_start(bqk_sb[:], bqk_e[:])
            bp_sb = p_cst.tile([128, 8], dt.float32)
            nc.sync.dma_start(bp_sb[:], bp_e[:])
            bvr_sb = p_cst.tile([1, CL], dt.float32r)
            nc.sync.dma_start(bvr_sb[:], bvr_e[:])

            # persistent: q^T rows 0:512 in qk_sb[0..3], k^T rows in qk_sb[4..7]
            qk_sb = [p_qk.tile([128, T], dt.float32r, tag="qk", name=f"qkt{i}") for i in range(8)]
            # v with interleaved ones column per head: [v_h(64) | 1] * 8 = 520 cols
            v_sb = [p_v.tile([128, 8 * 65], dt.float32r, tag="v", name=f"vt{i}") for i in range(NT)]

            # ---------------- Phase A: qkv ----------------
            with ExitStack() as actx:
                p_xt = actx.enter_context(tc.tile_pool(name="xt", bufs=12))
                p_w = actx.enter_context(tc.tile_pool(name="w", bufs=8))
                pp_qk = actx.enter_context(tc.tile_pool(name="ppqk", bufs=3, space="PSUM"))
                pp_v = actx.enter_context(tc.tile_pool(name="ppv", bufs=2, space="PSUM"))

                w_sb = []
                for c in range(8):
                    wt = p_w.tile([128, 1536], dt.float32r)
                    nc.sync.dma_start(wt[:, 0:1024], wqk_e[c * 128:(c + 1) * 128, :])
                    nc.sync.dma_start(wt[:, 1024:1536], wv_e[c * 128:(c + 1) * 128, :])
                    w_sb.append(wt)

                for n in range(NQ):
                    xts = []
                    for c in range(8):
                        xt = p_xt.tile([128, 512], dt.float32r)
                        nc.sync.dma_start(xt[:], xT_e[c * 128:(c + 1) * 128, n * 512:(n + 1) * 512])
                        xts.append(xt)
                    for mq in range(8):
                        ps_t = pp_qk.tile([128, 512], dt.float32)
                        for c in range(8):
                            nc.tensor.matmul(ps_t[:], w_sb[c][:, mq * 128:(mq + 1) * 128],
                                             xts[c][:], start=(c == 0), stop=(c == 7))
                        nc.scalar.activation(qk_sb[mq][:, n * 512:(n + 1) * 512], ps_t[:],
                                             AF.Identity, bias=bqk_sb[:, mq:mq + 1])
                    for tv in range(4):
                        ps_v = pp_v.tile([128, 512], dt.float32)
                        for c in range(8):
                            nc.tensor.matmul(ps_v[:], xts[c][:, tv * 128:(tv + 1) * 128],
                                             w_sb[c][:, 1024:1536], start=(c == 0), stop=False)
                        nc.tensor.matmul(ps_v[:], ones_row[:], bvr_sb[:], start=False, stop=True)
                        vt = v_sb[n * 4 + tv]
                        nc.scalar.activation(
                            vt[:].rearrange("p (h s) -> p h s", s=65)[:, :, 0:64],
                            ps_v[:].rearrange("p (h s) -> p h s", s=64),
                            AF.Copy)
                        nc.vector.tensor_copy(vt[:, 64:520:65], ones_f[:, 0:8])

            # ---------------- Phase B + C ----------------
            with ExitStack() as bctx:
                p_att = bctx.enter_context(tc.tile_pool(name="att", bufs=6))
                p_y = bctx.enter_context(tc.tile_pool(name="yt", bufs=6))
                p_rec = bctx.enter_context(tc.tile_pool(name="rec", bufs=2))
                p_bc = bctx.enter_context(tc.tile_pool(name="bc", bufs=2))
                p_out = bctx.enter_context(tc.tile_pool(name="osb", bufs=8))
                p_wp = bctx.enter_context(tc.tile_pool(name="wp", bufs=4))
                pp_wk = bctx.enter_context(tc.tile_pool(name="ppwk", bufs=4, space="PSUM"))
                pp_y = bctx.enter_context(tc.tile_pool(name="ppy", bufs=3, space="PSUM"))
                pp_bc = bctx.enter_context(tc.tile_pool(name="ppbc", bufs=1, space="PSUM"))
                p_dram = bctx.enter_context(tc.tile_pool(name="ccd", bufs=2, space="DRAM"))

                wp_sb = []
                for ci in range(4):
                    wpt = p_wp.tile([128, C], dt.float32r)
                    nc.sync.dma_start(wpt[:], wp_e[ci * 128:(ci + 1) * 128, :])
                    wp_sb.append(wpt)

                for n in range(NQ):
                    yt_tiles = []
                    for hp in range(4):
                        yt = p_y.tile([128, 512], dt.float32r)
                        yt_tiles.append(yt)
                        m_max = 4 * n + 4
                        h0, h1 = 2 * hp, 2 * hp + 1
                        y_pss = {h: pp_y.tile([128, 512], dt.float32, tag="ypsum", name=f"yps{h}") for h in (h0, h1)}
                        for m in range(m_max):
                            r = m - 4 * n  # >= 0 on the causal band
                            q_off = 128 * r if r >= 0 else 0
                            N = 512 - q_off
                            for h in (h0, h1):
                                base = (h % 2) * 64
                                qt = qk_sb[h // 2]
                                kt = qk_sb[4 + h // 2]
                                s_ps = pp_wk.tile([128, 512], dt.float32, tag="wk")
                                nc.tensor.matmul(
                                    s_ps[:, 0:N],
                                    kt[base:base + 64, m * 128:(m + 1) * 128],
                                    qt[base:base + 64, n * 512 + q_off:(n + 1) * 512],
                                    start=True, stop=True)
                                a_t = p_att.tile([128, 512], dt.float32r)
                                nc.scalar.activation(a_t[:, 0:N], s_ps[:, 0:N], AF.Exp, scale=float(SCALE))
                                if r >= 0:
                                    nc.gpsimd.affine_select(
                                        out=a_t[:, 0:128], in_=a_t[:, 0:128],
                                        compare_op=mybir.AluOpType.is_ge, fill=0.0, base=0,
                                        pattern=[[1, 128]], channel_multiplier=-1)
                                nc.tensor.matmul(
                                    y_pss[h][0:65, q_off:512],
                                    v_sb[m][:, h * 65:h * 65 + 65],
                                    a_t[:, 0:N],
                                    start=(m == 0), stop=(m == m_max - 1))
                        for h in (h0, h1):
                            base = (h % 2) * 64
                            rec = p_rec.tile([128, 512], dt.float32r)
                            nc.vector.reciprocal(rec[64:65, :], y_pss[h][64:65, :])
                            bc_ps = pp_bc.tile([64, 512], dt.float32)
                            nc.tensor.matmul(bc_ps[:], ones_rr[64:65, :], rec[64:65, :],
                                             start=True, stop=True)
                            bc_sb = p_bc.tile([128, 512], dt.float32)
                            nc.vector.tensor_copy(bc_sb[0:64, :], bc_ps[:])
                            nc.vector.tensor_mul(yt[base:base + 64, :], y_pss[h][0:64, :],
                                                 bc_sb[0:64, :])
                    # phase C for this query chunk
                    cc_in = p_dram.tile([C, 512], dt.float32, tag="ccin")
                    cc_out = p_dram.tile([CL, 512], dt.float32, tag="ccout")
                    for co in range(8):
                        o_ps = pp_wk.tile([128, 512], dt.float32, tag="wk")
                        for ci in range(4):
                            nc.tensor.matmul(o_ps[:], wp_sb[ci][:, co * 128:(co + 1) * 128],
                                             yt_tiles[ci][:], start=(ci == 0), stop=(ci == 3))
                        o_sb = p_out.tile([128, 512], dt.float32)
                        nc.scalar.activation(o_sb[:], o_ps[:], AF.Identity, bias=bp_sb[:, co:co + 1])
                        nc.sync.dma_start(cc_in[co * 128:(co + 1) * 128, :], o_sb[:])
                    nc.gpsimd.collective_compute(
                        "ReduceScatter", mybir.AluOpType.add,
                        ins=[cc_in[:]], outs=[cc_out[:]], replica_groups=RG)
                    nc.sync.dma_start(out_e[:, n * 512:(n + 1) * 512], cc_out[:])

    nc.finalize()
    return nc


def _get_nc():
    if "nc" not in _CACHE:
        _CACHE["nc"] = _build_nc()
    return _CACHE["nc"]


def _make_in_maps(x, W_attn, b_attn, W_proj, b_proj):
    x = np.asarray(x, dtype=np.float32)
    W_attn = np.asarray(W_attn, dtype=np.float32)
    b_attn = np.asarray(b_attn, dtype=np.float32)
    W_proj = np.asarray(W_proj, dtype=np.float32)
    b_proj = np.asarray(b_proj, dtype=np.float32)

    in_maps = []
    for core in range(8):
        b, hg = core // 2, core % 2
        lo, hi = hg * CL, (hg + 1) * CL
        wq = W_attn[:, lo:hi]
        wk = W_attn[:, C + lo:C + hi]
        wv = W_attn[:, 2 * C + lo:2 * C + hi]
        bq = b_attn[lo:hi]
        bk = b_attn[C + lo:C + hi]
        bv = b_attn[2 * C + lo:2 * C + hi]
        bp = b_proj if hg == 0 else np.zeros_like(b_proj)
        in_maps.append({
            "xT": np.ascontiguousarray(x[b].T),
            "wqk": np.ascontiguousarray(np.concatenate([wq, wk], axis=1)),
            "wv": np.ascontiguousarray(wv),
            "bqk": np.ascontiguousarray(np.concatenate([bq, bk]).reshape(8, 128).T),
            "bvr": np.ascontiguousarray(bv.reshape(1, CL)),
            "wproj": np.ascontiguousarray(W_proj[lo:hi, :]),
            "bproj": np.ascontiguousarray(bp.reshape(8, 128).T),
        })
    return in_maps


def _assemble(results):
    out = np.empty((B, T, C), dtype=np.float32)
    for b in range(B):
        top = results[2 * b]["out"]        # out^T rows 0:512
        bot = results[2 * b + 1]["out"]    # out^T rows 512:1024
        out[b] = np.concatenate([top, bot], axis=0).T
    return out


def run(trace=False, **inputs):
    nc = _get_nc()
    in_maps = _make_in_maps(**inputs)
    kw = {}
    if trace:
        kw = dict(trace=True, trace_cores=[0])
    res = run_bass_kernel_spmd(nc, in_maps, list(range(8)), **kw)
    return _assemble(res.results), res


def kernel(**inputs) -> np.ndarray:
    out, _ = run(trace=False, **inputs)
    return out


# revision 6
# speedup vs baseline: 1.0792x; 1.0792x over previous
"""Causal self-attention (B=4, T=2048, C=1024, H=16) on 8 TRN2 NeuronCores.

Sharding: core = 2*b + hg  (b = batch 0..3, hg = head-group 0..1, 8 heads each).
Each core computes, for its (b, hg):
  - qkv^T for its 8 heads via fp32r matmuls (x^T resident, W chunks stationary)
  - causal attention per head in scores-transposed orientation [keys, queries]
    with exp on ACT (no max subtraction; scores are O(1)), denominators via an
    appended ones-column in the AV matmul, and deferred normalization
  - partial output projection out^T = Wproj_loc^T @ y^T (+ bias on hg==0)
  - pairwise ReduceScatter (cores 2b, 2b+1) so each core returns half the
    out^T rows, already summed.
Host side transposes x per batch on the way in and reassembles/transposes the
output on the way out. All matmuls run in float32r (~1e-4 rel err, full PE rate).
"""
import numpy as np

import concourse.bass as bass
from concourse import bacc, mybir
from concourse.tile import TileContext
from concourse.bass_utils import run_bass_kernel_spmd

dt = mybir.dt
AF = mybir.ActivationFunctionType

B, T, C, H = 4, 2048, 1024, 16
D = 64              # head dim
HL = 8              # heads per core
CL = HL * D         # 512 local channels
NQ = T // 512       # 4 query chunks of 512
NT = T // 128       # 16 key/time chunks of 128
SCALE = 1.0 / np.sqrt(D)

_CACHE = {}


def _build_nc():
    nc = bacc.Bacc("TRN2", target_bir_lowering=False, debug=False)

    xT_e = nc.declare_dram_parameter("xT", [C, T], dt.float32r, isOutput=False)
    wqk_e = nc.declare_dram_parameter("wqk", [C, 2 * CL], dt.float32r, isOutput=False)
    wv_e = nc.declare_dram_parameter("wv", [C, CL], dt.float32r, isOutput=False)
    bqk_e = nc.declare_dram_parameter("bqk", [128, 8], dt.float32, isOutput=False)
    bvr_e = nc.declare_dram_parameter("bvr", [1, CL], dt.float32r, isOutput=False)
    wp_e = nc.declare_dram_parameter("wproj", [CL, C], dt.float32r, isOutput=False)
    bp_e = nc.declare_dram_parameter("bproj", [128, 8], dt.float32, isOutput=False)
    out_e = nc.declare_dram_parameter("out", [CL, T], dt.float32, isOutput=True)

    RG = [[0, 1], [2, 3], [4, 5], [6, 7]]

    with TileContext(nc) as tc, nc.allow_low_precision("fp32r intermediates by design"):
        from contextlib import ExitStack
        with ExitStack() as top:
            p_cst = top.enter_context(tc.tile_pool(name="cst", bufs=1))
            p_qk = top.enter_context(tc.tile_pool(name="qk", bufs=8))
            p_v = top.enter_context(tc.tile_pool(name="v", bufs=16))

            ones_f = p_cst.tile([128, 128], dt.float32)
            nc.gpsimd.memset(ones_f[:], 1.0)
            ones_row = p_cst.tile([1, 128], dt.float32r)
            nc.vector.tensor_copy(ones_row[:], ones_f[0:1, :])
            bqk_sb = p_cst.tile([128, 8], dt.float32)
            nc.sync.dma_start(bqk_sb[:], bqk_e[:])
            bp_sb = p_cst.tile([128, 8], dt.float32)
            nc.sync.dma_start(bp_sb[:], bp_e[:])
            bvr_sb = p_cst.tile([1, CL], dt.float32r)
            nc.sync.dma_start(bvr_sb[:], bvr_e[:])

            # persistent: q^T rows 0:512 in qk_sb[0..3], k^T rows in qk_sb[4..7]
            qk_sb = [p_qk.tile([128, T], dt.float32r, tag="qk", name=f"qkt{i}") for i in range(8)]
            # v with interleaved ones column per head: [v_h(64) | 1] * 8 = 520 cols
            v_sb = [p_v.tile([128, 8 * 65], dt.float32r, tag="v", name=f"vt{i}") for i in range(NT)]

            # ---------------- Phase A: qkv ----------------
            with ExitStack() as actx:
                p_xt = actx.enter_context(tc.tile_pool(name="xt", bufs=12))
                p_w = actx.enter_context(tc.tile_pool(name="w", bufs=8))
                pp_qk = actx.enter_context(tc.tile_pool(name="ppqk", bufs=3, space="PSUM"))
                pp_v = actx.enter_context(tc.tile_pool(name="ppv", bufs=2, space="PSUM"))

                w_sb = []
                for c in range(8):
                    wt = p_w.tile([128, 1536], dt.float32r)
                    nc.sync.dma_start(wt[:, 0:1024], wqk_e[c * 128:(c + 1) * 128, :])
                    nc.sync.dma_start(wt[:, 1024:1536], wv_e[c * 128:(c + 1) * 128, :])
                    w_sb.append(wt)

                for n in range(NQ):
                    xts = []
                    for c in range(8):
                        xt = p_xt.tile([128, 512], dt.float32r)
                        nc.sync.dma_start(xt[:], xT_e[c * 128:(c + 1) * 128, n * 512:(n + 1) * 512])
                        xts.append(xt)
                    for mq in range(8):
                        ps_t = pp_qk.tile([128, 512], dt.float32)
                        for c in range(8):
                            nc.tensor.matmul(ps_t[:], w_sb[c][:, mq * 128:(mq + 1) * 128],
                                             xts[c][:], start=(c == 0), stop=(c == 7))
                        nc.scalar.activation(qk_sb[mq][:, n * 512:(n + 1) * 512], ps_t[:],
                                             AF.Identity, bias=bqk_sb[:, mq:mq + 1])
                    for tv in range(4):
                        ps_v = pp_v.tile([128, 512], dt.float32)
                        for c in range(8):
                            nc.tensor.matmul(ps_v[:], xts[c][:, tv * 128:(tv + 1) * 128],
                                             w_sb[c][:, 1024:1536], start=(c == 0), stop=False)
                        nc.tensor.matmul(ps_v[:], ones_row[:], bvr_sb[:], start=False, stop=True)
                        vt = v_sb[n * 4 + tv]
                        nc.scalar.activation(
                            vt[:].rearrange("p (h s) -> p h s", s=65)[:, :, 0:64],
                            ps_v[:].rearrange("p (h s) -> p h s", s=64),
                            AF.Copy)
                        nc.vector.tensor_copy(vt[:, 64:520:65], ones_f[:, 0:8])

            # ---------------- Phase B + C ----------------
            with ExitStack() as bctx:
                p_att = bctx.enter_context(tc.tile_pool(name="att", bufs=4))
                p_y = bctx.enter_context(tc.tile_pool(name="yt", bufs=6))
                p_rec = bctx.enter_context(tc.tile_pool(name="rec", bufs=2))
                p_bc = bctx.enter_context(tc.tile_pool(name="bc", bufs=2))
                p_out = bctx.enter_context(tc.tile_pool(name="osb", bufs=8))
                p_wp = bctx.enter_context(tc.tile_pool(name="wp", bufs=4))
                pp_wk = bctx.enter_context(tc.tile_pool(name="ppwk", bufs=2, space="PSUM"))
                pp_y = bctx.enter_context(tc.tile_pool(name="ppy", bufs=3, space="PSUM"))
                pp_bc = bctx.enter_context(tc.tile_pool(name="ppbc", bufs=1, space="PSUM"))
                p_dram = bctx.enter_context(tc.tile_pool(name="ccd", bufs=2, space="DRAM"))

                wp_sb = []
                for ci in range(4):
                    wpt = p_wp.tile([128, C], dt.float32r)
                    nc.sync.dma_start(wpt[:], wp_e[ci * 128:(ci + 1) * 128, :])
                    wp_sb.append(wpt)

                for n in range(NQ):
                    yt_tiles = []
                    for hp in range(4):
                        yt = p_y.tile([128, 512], dt.float32r)
                        yt_tiles.append(yt)
                        m_max = 4 * n + 4
                        h0, h1 = 2 * hp, 2 * hp + 1
                        y_pss = {h: pp_y.tile([128, 512], dt.float32, tag="ypsum", name=f"yps{h}") for h in (h0, h1)}
                        # process key-chunks in pairs: one 2-bank score tile +
                        # one exp per (pair, head)
                        for j in range(m_max // 2):
                            m0, m1 = 2 * j, 2 * j + 1
                            r0, r1 = m0 - 4 * n, m1 - 4 * n
                            q0 = 128 * r0 if r0 >= 0 else 0   # m0 valid q start
                            q1 = 128 * r1 if r1 >= 0 else 0
                            for h in (h0, h1):
                                base = (h % 2) * 64
                                qt = qk_sb[h // 2]
                                kt = qk_sb[4 + h // 2]
                                s_ps = pp_wk.tile([128, 1024], dt.float32, tag="wk")
                                nc.tensor.matmul(
                                    s_ps[:, q0:512],
                                    kt[base:base + 64, m0 * 128:(m0 + 1) * 128],
                                    qt[base:base + 64, n * 512 + q0:(n + 1) * 512],
                                    start=True, stop=True)
                                nc.tensor.matmul(
                                    s_ps[:, 512 + q1:1024],
                                    kt[base:base + 64, m1 * 128:(m1 + 1) * 128],
                                    qt[base:base + 64, n * 512 + q1:(n + 1) * 512],
                                    start=True, stop=True)
                                a_t = p_att.tile([128, 1024], dt.float32r)
                                # one exp covering both halves (unwritten gap
                                # columns produce garbage that is never read)
                                nc.scalar.activation(a_t[:, q0:1024], s_ps[:, q0:1024],
                                                     AF.Exp, scale=float(SCALE))
                                if r0 >= 0:
                                    nc.gpsimd.affine_select(
                                        out=a_t[:, q0:q0 + 128], in_=a_t[:, q0:q0 + 128],
                                        compare_op=mybir.AluOpType.is_ge, fill=0.0, base=0,
                                        pattern=[[1, 128]], channel_multiplier=-1)
                                if r1 >= 0:
                                    nc.gpsimd.affine_select(
                                        out=a_t[:, 512 + q1:512 + q1 + 128],
                                        in_=a_t[:, 512 + q1:512 + q1 + 128],
                                        compare_op=mybir.AluOpType.is_ge, fill=0.0, base=0,
                                        pattern=[[1, 128]], channel_multiplier=-1)
                                nc.tensor.matmul(
                                    y_pss[h][0:65, q0:512],
                                    v_sb[m0][:, h * 65:h * 65 + 65],
                                    a_t[:, q0:512],
                                    start=(m0 == 0), stop=False)
                                nc.tensor.matmul(
                                    y_pss[h][0:65, q1:512],
                                    v_sb[m1][:, h * 65:h * 65 + 65],
                                    a_t[:, 512 + q1:1024],
                                    start=False, stop=(m1 == m_max - 1))
                        for h in (h0, h1):
                            base = (h % 2) * 64
                            rec_s = p_rec.tile([128, 512], dt.float32, tag="recs")
                            rec = p_rec.tile([128, 512], dt.float32, tag="rec")
                            # custom-DVE ops require partition base 0: move the
                            # denominator row down first (cross-base copy).
                            nc.vector.tensor_copy(rec_s[0:1, :], y_pss[h][64:65, :])
                            nc.vector.reciprocal_approx_fast(out=rec[0:1, :], in_=rec_s[0:1, :])
                            bc_ps = pp_bc.tile([64, 512], dt.float32)
                            nc.tensor.matmul(bc_ps[:], ones_f[0:1, 0:64], rec[0:1, :],
                                             start=True, stop=True)
                            bc_sb = p_bc.tile([128, 512], dt.float32)
                            nc.vector.tensor_copy(bc_sb[0:64, :], bc_ps[:])
                            nc.vector.tensor_mul(yt[base:base + 64, :], y_pss[h][0:64, :],
                                                 bc_sb[0:64, :])
                    # phase C for this query chunk
                    cc_in = p_dram.tile([C, 512], dt.float32, tag="ccin")
                    cc_out = p_dram.tile([CL, 512], dt.float32, tag="ccout")
                    for co in range(8):
                        o_ps = pp_wk.tile([128, 512], dt.float32, tag="wk")
                        for ci in range(4):
                            nc.tensor.matmul(o_ps[:], wp_sb[ci][:, co * 128:(co + 1) * 128],
                                             yt_tiles[ci][:], start=(ci == 0), stop=(ci == 3))
                        o_sb = p_out.tile([128, 512], dt.float32)
                        nc.scalar.activation(o_sb[:], o_ps[:], AF.Identity, bias=bp_sb[:, co:co + 1])
                        nc.sync.dma_start(cc_in[co * 128:(co + 1) * 128, :], o_sb[:])
                    nc.gpsimd.collective_compute(
                        "ReduceScatter", mybir.AluOpType.add,
                        ins=[cc_in[:]], outs=[cc_out[:]], replica_groups=RG)
                    nc.sync.dma_start(out_e[:, n * 512:(n + 1) * 512], cc_out[:])

    nc.finalize()
    return nc


def _get_nc():
    if "nc" not in _CACHE:
        _CACHE["nc"] = _build_nc()
    return _CACHE["nc"]


def _make_in_maps(x, W_attn, b_attn, W_proj, b_proj):
    x = np.asarray(x, dtype=np.float32)
    W_attn = np.asarray(W_attn, dtype=np.float32)
    b_attn = np.asarray(b_attn, dtype=np.float32)
    W_proj = np.asarray(W_proj, dtype=np.float32)
    b_proj = np.asarray(b_proj, dtype=np.float32)

    in_maps = []
    for core in range(8):
        b, hg = core // 2, core % 2
        lo, hi = hg * CL, (hg + 1) * CL
        wq = W_attn[:, lo:hi]
        wk = W_attn[:, C + lo:C + hi]
        wv = W_attn[:, 2 * C + lo:2 * C + hi]
        bq = b_attn[lo:hi]
        bk = b_attn[C + lo:C + hi]
        bv = b_attn[2 * C + lo:2 * C + hi]
        bp = b_proj if hg == 0 else np.zeros_like(b_proj)
        in_maps.append({
            "xT": np.ascontiguousarray(x[b].T),
            "wqk": np.ascontiguousarray(np.concatenate([wq, wk], axis=1)),
            "wv": np.ascontiguousarray(wv),
            "bqk": np.ascontiguousarray(np.concatenate([bq, bk]).reshape(8, 128).T),
            "bvr": np.ascontiguousarray(bv.reshape(1, CL)),
            "wproj": np.ascontiguousarray(W_proj[lo:hi, :]),
            "bproj": np.ascontiguousarray(bp.reshape(8, 128).T),
        })
    return in_maps


def _assemble(results):
    out = np.empty((B, T, C), dtype=np.float32)
    for b in range(B):
        top = results[2 * b]["out"]        # out^T rows 0:512
        bot = results[2 * b + 1]["out"]    # out^T rows 512:1024
        out[b] = np.concatenate([top, bot], axis=0).T
    return out


def run(trace=False, **inputs):
    nc = _get_nc()
    in_maps = _make_in_maps(**inputs)
    kw = {}
    if trace:
        kw = dict(trace=True, trace_cores=[0])
    res = run_bass_kernel_spmd(nc, in_maps, list(range(8)), **kw)
    return _assemble(res.results), res


def kernel(**inputs) -> np.ndarray:
    out, _ = run(trace=False, **inputs)
    return out
